# revision 7
# baseline (speedup 1.0000x reference)
"""Multi-head attention (axis-swapped variant) on 8 Trainium2 NeuronCores, v6.

Major changes over v5:
- Q/K projections computed as fp8(e4m3) DoubleRow matmuls (4x fewer PE
  cycles): host supplies x and Wq/Wk pre-quantized, with W columns
  permuted so each head's 8-dim slab lands on a 32-aligned partition
  base for its processing slot.
- Scores matmuls in fp8 DoubleRow (2x fewer PE cycles): Q/K stored as
  [128, 2(ktile), seq] fp8; misaligned head slabs served by shifted
  copies built with partition-strided SBUF->SBUF DMAs (no PE cost).
- exp split across ACT and DVE: ACT does exact Exp activations for most
  spans; DVE approximates exp for the rest via the Schraudolph
  int16-bitcast trick (tensor_scalar f32 -> int16 bits of the bf16
  result), validated bit-exact against the host model on the backend.
- Causal masks on Pool (gpsimd affine_select) for every head; Pool has
  no PSUM port so it carries no other work.
- PSUM evictions split between ACT (idle pre-attention / post-exp) and
  DVE by static tables.
"""

import numpy as np
import ml_dtypes

import concourse.bass as bass
import concourse.mybir as mybir
import concourse.tile as tile
from concourse.bass_utils import run_bass_kernel_spmd

F32 = mybir.dt.float32
BF16 = mybir.dt.bfloat16
FP8 = mybir.dt.float8e4
I16 = mybir.dt.int16
FP8NP = ml_dtypes.float8_e4m3

EMB = 1024
SEQ = 1024
BATCH = 2
NG = 4
HPG = 16
DH = 16
GCOLS = HPG * DH
DR = mybir.MatmulPerfMode.DoubleRow

SPAN = 1536
NJB = 8

SIZES = [SEQ - 128 * jb for jb in range(NJB)]
SPAN_GROUPS = [[1, 3], [0, 4], [2, 5, 6, 7]]
assert all(sum(SIZES[jb] for jb in g) == SPAN for g in SPAN_GROUPS)
SPAN_OF = {}
ORDER_POS = {}
_pos = 0
for _s, _g in enumerate(SPAN_GROUPS):
    _off = 0
    for _jb in _g:
        SPAN_OF[_jb] = (_s, _off)
        ORDER_POS[_jb] = _pos
        _pos += 1
        _off += SIZES[_jb]

# Schraudolph exp constants (bf16 bit-space), softmax scale 0.25 folded in.
ACONST = float(2.0 ** 7 / np.log(2.0) * 0.25)
BCONST = float(127.0 * 2 ** 7 - 0.0579 * 2 ** 7)

# ---- tuning tables -------------------------------------------------------
# exp engine per span index 3*h+s: 'A' = ACT exact, 'D' = DVE schraudolph.
EXP_COUNTS = {"A": 34, "D": 14}


def _exp_table():
    assert sum(EXP_COUNTS.values()) == 48
    out = []
    got = {k: 0 for k in EXP_COUNTS}
    for j in range(48):
        best, bestd = None, None
        for k in EXP_COUNTS:
            d = EXP_COUNTS[k] * (j + 1) / 48 - got[k]
            if bestd is None or d > bestd:
                best, bestd = k, d
        out.append(best)
        got[best] += 1
    return out


EXP_ENG = _exp_table()
QK_EVICT = ["A", "A", "D", "A", "A", "A", "D", "A"]      # 8 x [128,512]
V_EVICT = ["D"] * 16                                      # 16 x [128,128]
CN0_EVICT = ["D"] * 8                                     # rb0 transposes
CN1_EVICT = ["A", "D"] * 4                                # rb1 (tail)
PASS0_EVICT = ["D"] * 16                                  # 16 x [128,512]
PASS1_EVICT = ["A", "D"] * 4                              # 8 x [128,1024]


def base_p(slot):
    """Q8/K8 partition base of the head processed in `slot` (0..15)."""
    return 32 * (slot % 4) + 8 * (slot // 4)


def split_excess_waits(nc, cap=1):
    def fix_block(bb, dummy):
        insts = bb.instructions
        i = 0
        while i < len(insts):
            inst = insts[i]
            si = inst.sync_info
            waits = list(si.on_wait) if si is not None and si.on_wait else []
            if len(waits) > cap:
                eng = nc.engines[inst.engine]
                excess, keep = waits[:-cap], waits[-cap:]
                si.on_wait = keep
                pos = i
                for j in range(0, len(excess), cap):
                    chunk = excess[j : j + cap]
                    ev = eng.wait_ge(dummy, 1)
                    cur_list = nc.cur_bb.bb.instructions
                    assert cur_list[-1] is ev.ins
                    cur_list.pop()
                    ev.ins.sync_info.on_wait = chunk
                    insts.insert(pos, ev.ins)
                    pos += 1
                    i += 1
            i += 1

    with nc.semaphore("waitfix_dummy") as dummy:
        for f in nc.m.functions:
            for bb in f.blocks:
                fix_block(bb, dummy)


def build_nc():
    nc = bass.Bass()
    xT_d = nc.declare_dram_parameter("xT", [8, 128, SEQ], BF16, isOutput=False)
    x8_d = nc.declare_dram_parameter("x8", [4, 128, 2, SEQ], FP8, isOutput=False)
    wq8_d = nc.declare_dram_parameter("wq8", [4, 128, 2, 256], FP8, isOutput=False)
    wk8_d = nc.declare_dram_parameter("wk8", [4, 128, 2, 256], FP8, isOutput=False)
    wv_d = nc.declare_dram_parameter("wv", [8, 128, GCOLS], BF16, isOutput=False)
    wo_d = nc.declare_dram_parameter("wo", [2, 128, EMB], BF16, isOutput=False)
    id_d = nc.declare_dram_parameter("ident", [128, 128], BF16, isOutput=False)
    y0_d = nc.declare_dram_parameter("y0", [8, 128, EMB], BF16, isOutput=True)
    y1_d = nc.declare_dram_parameter("y1", [8, 128, EMB], BF16, isOutput=True)

    with tile.TileContext(nc) as tc:
        with (
            tc.tile_pool(name="big", bufs=1) as big,
            tc.tile_pool(name="atp", bufs=1) as atp,
            tc.tile_pool(name="yst", bufs=8) as yst,
            tc.tile_pool(name="scp", bufs=2, space="PSUM") as scp,
            tc.tile_pool(name="ctxp", bufs=1, space="PSUM") as ctxp,
        ):
            # ---- SBUF ----
            XTk = [big.tile([128, SEQ], BF16, name=f"xt{k}") for k in range(8)]
            X8 = [big.tile([128, 2, SEQ], FP8, name=f"x8_{k}") for k in range(4)]
            WQ8 = big.tile([128, 4, 2, 256], FP8)
            WK8 = big.tile([128, 4, 2, 256], FP8)
            WV = big.tile([128, 8, GCOLS], BF16)
            WO = big.tile([128, 2, EMB], BF16)
            IDT = big.tile([128, 128], BF16)
            Q8v = [big.tile([128, 2, SEQ], FP8, name=f"q8v{v}") for v in range(4)]
            K8v = [big.tile([128, 2, SEQ], FP8, name=f"k8v{v}") for v in range(4)]
            VAr = [big.tile([128, 8, 8, 17], BF16, name=f"va{r}") for r in range(2)]
            CTr = [big.tile([128, 8, 128], BF16, name=f"ct{r}") for r in range(2)]
            CNr = [[big.tile([128, 128], BF16, name=f"cn{r}i{i}")
                    for i in range(8)] for r in range(2)]
            Y0 = big.tile([128, 8, EMB], BF16)
            JNK = big.tile([128, 512], BF16)
            AT = [[atp.tile([128, SPAN], BF16, name=f"at{i}s{s}")
                   for s in range(3)] for i in range(4)]

            scn = [0]

            def sc_tile(shape=None, dtype=F32, tag="sc"):
                scn[0] += 1
                pool = scp if tag == "sc" else ctxp
                return pool.tile(shape or [128, SPAN], dtype, tag=tag,
                                 name=f"{tag}{scn[0]}")

            def evict_copy(code, dst, src):
                if code == "A":
                    nc.scalar.copy(dst, src)
                else:
                    nc.vector.tensor_copy(dst, src)

            # ---- warm-up: ramp the PE pstate while DMAs stream ----
            nc.gpsimd.memset(JNK[:], 1.0)
            dmy = sc_tile([128, 512], F32, tag="tp")
            for _ in range(4):
                nc.tensor.matmul(dmy[:], JNK[:, 0:128], JNK[:],
                                 start=True, stop=True)

            # ---- DMA in: what gates Q/K projections first ----
            nc.sync.dma_start(WQ8[:], wq8_d[:].rearrange("k p a m -> p k a m"))
            nc.sync.dma_start(WK8[:], wk8_d[:].rearrange("k p a m -> p k a m"))
            for kbp in range(4):
                nc.sync.dma_start(X8[kbp][:], x8_d[kbp])
            nc.sync.dma_start(IDT[:], id_d[:])
            for r in range(2):
                nc.gpsimd.memset(VAr[r][:, :, :, 16:17], 1.0)

            # ---- Q/K projections: fp8 DoubleRow, contraction 256/ktile-pair
            evn = [0]

            def proj_qk(W8, DST, tags, ic_outer=False):
                combos = ([(t, ic) for ic in range(2) for t in range(2)]
                          if ic_outer else
                          [(t, ic) for t in range(2) for ic in range(2)])
                for t, ic in combos:
                    tg = tags[evn[0] % len(tags)]
                    if tg == "sc":
                        p = sc_tile()[:, 0:512]
                    else:
                        p = sc_tile([128, 512], F32, tag=tg)[:]
                    for kbp in range(4):
                        nc.tensor.matmul(
                            p, W8[:, kbp, :, 128 * t : 128 * t + 128],
                            X8[kbp][:, :, 512 * ic : 512 * ic + 512],
                            start=(kbp == 0), stop=(kbp == 3),
                            perf_mode=DR)
                    base = DST[:, t, 512 * ic : 512 * ic + 512]
                    # split the eviction across both engines
                    nc.scalar.copy(base[:, 0:256], p[:, 0:256])
                    nc.vector.tensor_copy(base[:, 256:512], p[:, 256:512])
                    evn[0] += 1

            proj_qk(WQ8, Q8v[0], ["sc"])
            proj_qk(WK8, K8v[0], ["tp", "ctx"], ic_outer=True)

            # ---- remaining input DMAs (no waits; keep SP queue flowing) ----
            for kb in range(8):
                nc.sync.dma_start(XTk[kb][:], xT_d[kb, :, :])
            nc.sync.dma_start(WV[:], wv_d[:].rearrange("k p n -> p k n"))
            nc.sync.dma_start(WO[:], wo_d[:].rearrange("r p n -> p r n"))

            # ---- shifted variants via partition-strided SBUF->SBUF DMA ----
            # (emitted after all input DMAs: their eviction waits would
            # head-of-line block the SP queue otherwise)
            for (SRC, DSTS) in ((Q8v[0], Q8v), (K8v[0], K8v)):
                s_u8 = SRC[:].bitcast(mybir.dt.uint8)
                for v in range(1, 4):
                    d_u8 = DSTS[v][:].bitcast(mybir.dt.uint8)
                    for k in range(4):
                        nc.sync.dma_start(
                            d_u8[32 * k : 32 * k + 8, :],
                            s_u8[32 * k + 8 * v : 32 * k + 8 * v + 8, :])

            # ---- deferred unit queue (popped between span exps) ----
            vcnt = [0]

            def proj_v_unit(mt, half):
                def f():
                    p = sc_tile([128, 512], F32, tag="tp")[:, 0:128]
                    for kb in range(8):
                        nc.tensor.matmul(
                            p, XTk[kb][:, 128 * mt : 128 * mt + 128],
                            WV[:, kb, 128 * half : 128 * half + 128],
                            start=(kb == 0), stop=(kb == 7))
                    code = V_EVICT[vcnt[0]]
                    vcnt[0] += 1
                    evict_copy(code,
                               VAr[half][:, mt, :, 0:16],
                               p.rearrange("p (h e) -> p h e", e=16))
                return f

            def transpose_unit(rb):
                def f():
                    TPb = sc_tile([128, 8, 128], BF16, tag="tp")
                    for ib in range(8):
                        nc.tensor.transpose(
                            TPb[:, ib, :], CTr[rb][:, ib, :], IDT[:])
                        evict_copy(CN0_EVICT[ib] if rb == 0 else CN1_EVICT[ib],
                                   CNr[rb][ib][:], TPb[:, ib, :])
                return f

            p0cnt = [0]

            def pass0_unit(ib, ic):
                def f():
                    tg = "tp" if (2 * ib + ic) % 2 == 0 else "ctx"
                    yp = sc_tile([128, 512], F32, tag=tg)[:]
                    nc.tensor.matmul(
                        yp, CNr[0][ib][:],
                        WO[:, 0, 512 * ic : 512 * ic + 512],
                        start=True, stop=True)
                    code = PASS0_EVICT[p0cnt[0]]
                    p0cnt[0] += 1
                    evict_copy(code,
                               Y0[:, ib, 512 * ic : 512 * ic + 512], yp)
                    nc.sync.dma_start(
                        y0_d[ib, :, 512 * ic : 512 * ic + 512],
                        Y0[:, ib, 512 * ic : 512 * ic + 512])
                return f

            units = []  # (min_head, thunk)
            for mt in range(8):
                units.append((0, proj_v_unit(mt, 0)))
            for mt in range(8):
                units.append((4, proj_v_unit(mt, 1)))
            units.append((9, transpose_unit(0)))
            for ib in range(8):
                for ic in range(2):
                    units.append((9, pass0_unit(ib, ic)))

            def head_slices(h, qk):
                v = h // 4
                b = 32 * (h % 4)
                return (Q8v if qk == "q" else K8v)[v], b

            # ---- attention ----
            def emit_mask(A, jb):
                o = SPAN_OF[jb][1]
                nc.gpsimd.affine_select(
                    out=A[:, o : o + 128], in_=A[:, o : o + 128],
                    compare_op=mybir.AluOpType.is_ge,
                    fill=0.0, base=0, pattern=[[1, 128]],
                    channel_multiplier=-1)

            def span_mms(h, s, sct):
                QT, b = head_slices(h, "q")
                KT, _ = head_slices(h, "k")
                for jb in SPAN_GROUPS[s]:
                    size = SIZES[jb]
                    i0 = 128 * jb
                    off = SPAN_OF[jb][1]
                    done = 0
                    while done < size:
                        # never cross a 512-f32 PSUM bank boundary
                        cw = min(512 - (off + done) % 512, size - done)
                        nc.tensor.matmul(
                            sct[:, off + done : off + done + cw],
                            KT[b : b + 8, :, i0 : i0 + 128],
                            QT[b : b + 8, :, i0 + done : i0 + done + cw],
                            start=True, stop=True,
                            perf_mode=DR, tile_position=(b, 0))
                        done += cw

            def span_exp(h, s, sct):
                A = AT[h % 4][s]
                if h == HPG - 1:
                    nc.scalar.activation(
                        A[:, 0:768], sct[:, 0:768],
                        mybir.ActivationFunctionType.Exp, scale=0.25)
                    nc.vector.tensor_scalar(
                        A[:, 768:SPAN].bitcast(I16), sct[:, 768:SPAN],
                        ACONST, BCONST,
                        mybir.AluOpType.mult, mybir.AluOpType.add)
                elif EXP_ENG[3 * h + s] == "A":
                    nc.scalar.activation(
                        A[:], sct[:],
                        mybir.ActivationFunctionType.Exp, scale=0.25)
                else:
                    nc.vector.tensor_scalar(
                        A[:].bitcast(I16), sct[:], ACONST, BCONST,
                        mybir.AluOpType.mult, mybir.AluOpType.add)
                for jb in SPAN_GROUPS[s]:
                    emit_mask(A, jb)

            def scores_and_exp(h):
                t0 = sc_tile()
                span_mms(h, 0, t0)
                t1 = sc_tile()
                span_mms(h, 1, t1)
                span_exp(h, 0, t0)
                t2 = sc_tile()
                span_mms(h, 2, t2)
                span_exp(h, 1, t1)
                for _ in range(4 if h <= 1 else 3):
                    if units and units[0][0] <= h:
                        units.pop(0)[1]()
                span_exp(h, 2, t2)

            def ctx_head(h):
                rb, hh = divmod(h, 8)
                CTX = ctxp.tile([128, 8, 17], F32, tag="ctx", name=f"ctx{h}")
                for ib in range(8):
                    # diag last so ctx starts before this head's masks finish
                    contribs = sorted((jb for jb in range(ib + 1)),
                                      key=lambda jb: ORDER_POS[jb])
                    if ib in contribs:
                        contribs.remove(ib)
                        contribs.append(ib)
                    for idx, jb in enumerate(contribs):
                        s, off = SPAN_OF[jb]
                        o = off + 128 * (ib - jb)
                        nc.tensor.matmul(
                            CTX[:, ib, :], AT[h % 4][s][:, o : o + 128],
                            VAr[rb][:, jb, hh, :],
                            start=(idx == 0), stop=(idx == len(contribs) - 1),
                            skip_group_check=True)
                R3 = yst.tile([128, 8], F32, tag="r3", bufs=2)
                nc.vector.reciprocal(
                    R3[:], CTX[:, :, 16:17].rearrange("p a b -> p (a b)"))
                nc.vector.tensor_mul(
                    CTr[rb][:, :, 16 * hh : 16 * hh + 16],
                    CTX[:, :, 0:16],
                    R3[:].unsqueeze(2).broadcast_to([128, 8, 16]))

            # ---- main loop ----
            for h in range(HPG):
                if h >= 2:
                    ctx_head(h - 2)
                scores_and_exp(h)
            ctx_head(HPG - 2)
            ctx_head(HPG - 1)

            # ---- out-projection pass 1 (ctx cols 128-255 partial),
            # fused per-ib: transpose -> CN evict -> matmuls -> Y evict ----
            TPc = sc_tile([128, 8, 128], BF16, tag="tp")
            for ib in range(8):
                nc.tensor.transpose(TPc[:, ib, :], CTr[1][:, ib, :], IDT[:])
                evict_copy(CN1_EVICT[ib], CNr[1][ib][:], TPc[:, ib, :])
                yp = sc_tile()[:, 0:1024]
                for ic in range(2):
                    nc.tensor.matmul(
                        yp[:, 512 * ic : 512 * ic + 512],
                        CNr[1][ib][:],
                        WO[:, 1, 512 * ic : 512 * ic + 512],
                        start=True, stop=True)
                Y = yst.tile([128, 1024], BF16, tag="y", bufs=8)
                nc.scalar.copy(Y[:, 0:512], yp[:, 0:512])
                nc.vector.tensor_copy(Y[:, 512:1024], yp[:, 512:1024])
                nc.sync.dma_start(y1_d[ib, :, :], Y[:])

    split_excess_waits(nc)
    return nc


_NC_CACHE = None


def _get_nc():
    global _NC_CACHE
    if _NC_CACHE is None:
        _NC_CACHE = build_nc()
    return _NC_CACHE


def _bf(a):
    return np.ascontiguousarray(a).astype(ml_dtypes.bfloat16)


def _f8(a):
    return np.ascontiguousarray(a).astype(FP8NP)


def _colmap():
    """cm[128*t + m] = original column (within the 256-col group) stored
    at stationary free position m of chunk t, per the slot partition map."""
    cm = np.zeros(256, dtype=np.int64)
    for t in range(2):
        for m in range(128):
            v = (m % 32) // 8
            k = m // 32
            slot = 4 * v + k
            d = m % 8
            cm[128 * t + m] = 16 * slot + 8 * t + d
    return cm


_CM = _colmap()


def kernel(x, Wq, Wk, Wv, Wo, bo):
    x = np.asarray(x, dtype=np.float32)
    Wq = np.asarray(Wq, dtype=np.float32)
    Wk = np.asarray(Wk, dtype=np.float32)
    Wv = np.asarray(Wv, dtype=np.float32)
    Wo = np.asarray(Wo, dtype=np.float32)
    bo = np.asarray(bo, dtype=np.float32)

    nc = _get_nc()
    ident = np.eye(128, dtype=np.float32)

    in_maps = []
    for c in range(8):
        b, g = divmod(c, NG)
        cols = slice(GCOLS * g, GCOLS * g + GCOLS)
        xT = x[b].T  # [emb, seq]
        x8 = xT.reshape(4, 2, 128, SEQ).transpose(0, 2, 1, 3)
        # wq8[kbp, p, kt, 128*t+m] = W[256*kbp + 128*kt + p, g_base + cm[...]]
        wq8 = Wq[:, cols][:, _CM].reshape(4, 2, 128, 256).transpose(0, 2, 1, 3)
        wk8 = Wk[:, cols][:, _CM].reshape(4, 2, 128, 256).transpose(0, 2, 1, 3)
        in_maps.append({
            "xT": _bf(xT.reshape(8, 128, SEQ)),
            "x8": _f8(x8),
            "wq8": _f8(wq8),
            "wk8": _f8(wk8),
            "wv": _bf(Wv[:, cols].reshape(8, 128, GCOLS)),
            "wo": _bf(Wo[cols, :].reshape(2, 128, EMB)),
            "ident": _bf(ident),
        })

    res = run_bass_kernel_spmd(nc, in_maps, core_ids=list(range(8)))
    out = np.zeros((BATCH, SEQ, EMB), dtype=np.float32)
    for c in range(8):
        b = c // NG
        out[b] += res.results[c]["y0"].reshape(SEQ, EMB).astype(np.float32)
        out[b] += res.results[c]["y1"].reshape(SEQ, EMB).astype(np.float32)
    out += bo[None, None, :]
    return out


# revision 9
# speedup vs baseline: 1.0318x; 1.0318x over previous
"""Multi-head attention (axis-swapped variant) on 8 Trainium2 NeuronCores, v6.

Major changes over v5:
- Q/K projections computed as fp8(e4m3) DoubleRow matmuls (4x fewer PE
  cycles): host supplies x and Wq/Wk pre-quantized, with W columns
  permuted so each head's 8-dim slab lands on a 32-aligned partition
  base for its processing slot.
- Scores matmuls in fp8 DoubleRow (2x fewer PE cycles): Q/K stored as
  [128, 2(ktile), seq] fp8; misaligned head slabs served by shifted
  copies built with partition-strided SBUF->SBUF DMAs (no PE cost).
- exp split across ACT and DVE: ACT does exact Exp activations for most
  spans; DVE approximates exp for the rest via the Schraudolph
  int16-bitcast trick (tensor_scalar f32 -> int16 bits of the bf16
  result), validated bit-exact against the host model on the backend.
- Causal masks on Pool (gpsimd affine_select) for every head; Pool has
  no PSUM port so it carries no other work.
- PSUM evictions split between ACT (idle pre-attention / post-exp) and
  DVE by static tables.
"""

import numpy as np
import ml_dtypes

import concourse.bass as bass
import concourse.mybir as mybir
import concourse.tile as tile
from concourse.bass_utils import run_bass_kernel_spmd

F32 = mybir.dt.float32
BF16 = mybir.dt.bfloat16
FP8 = mybir.dt.float8e4
I16 = mybir.dt.int16
FP8NP = ml_dtypes.float8_e4m3

EMB = 1024
SEQ = 1024
BATCH = 2
NG = 4
HPG = 16
DH = 16
GCOLS = HPG * DH
DR = mybir.MatmulPerfMode.DoubleRow

SPAN = 1536
NJB = 8

SIZES = [SEQ - 128 * jb for jb in range(NJB)]
SPAN_GROUPS = [[1, 3], [0, 4], [2, 5, 6, 7]]
assert all(sum(SIZES[jb] for jb in g) == SPAN for g in SPAN_GROUPS)
SPAN_OF = {}
ORDER_POS = {}
_pos = 0
for _s, _g in enumerate(SPAN_GROUPS):
    _off = 0
    for _jb in _g:
        SPAN_OF[_jb] = (_s, _off)
        ORDER_POS[_jb] = _pos
        _pos += 1
        _off += SIZES[_jb]

# Schraudolph exp constants (bf16 bit-space), softmax scale 0.25 folded in.
ACONST = float(2.0 ** 7 / np.log(2.0) * 0.25)
BCONST = float(127.0 * 2 ** 7 - 0.0579 * 2 ** 7)

# ---- tuning tables -------------------------------------------------------
# exp engine per span index 3*h+s: 'A' = ACT exact, 'D' = DVE schraudolph.
EXP_COUNTS = {"A": 31, "D": 17}


def _exp_table():
    assert sum(EXP_COUNTS.values()) == 48
    out = []
    got = {k: 0 for k in EXP_COUNTS}
    for j in range(48):
        best, bestd = None, None
        for k in EXP_COUNTS:
            d = EXP_COUNTS[k] * (j + 1) / 48 - got[k]
            if bestd is None or d > bestd:
                best, bestd = k, d
        out.append(best)
        got[best] += 1
    return out


EXP_ENG = _exp_table()
QK_EVICT = ["A", "A", "D", "A", "A", "A", "D", "A"]      # 8 x [128,512]
V_EVICT = ["D"] * 16                                      # 16 x [128,128]
CN0_EVICT = ["A", "D"] * 4                                # rb0 transposes
CN1_EVICT = ["A", "D"] * 4                                # rb1 (tail)
PASS0_EVICT = ["A", "D"] * 8                              # 16 x [128,512]
PASS1_EVICT = ["A", "D"] * 4                              # 8 x [128,1024]
K_IC_OUTER = False       # K projection ic-major (scores start earlier)
SPLIT_LAST_EXP = False   # last head's exps split across ACT+DVE
FUSED_TAIL = False       # per-ib transpose+CN+pass1 fusion
WARMUP_N = 5            # dummy warmup matmuls


def base_p(slot):
    """Q8/K8 partition base of the head processed in `slot` (0..15)."""
    return 32 * (slot % 4) + 8 * (slot // 4)


def split_excess_waits(nc, cap=1):
    def fix_block(bb, dummy):
        insts = bb.instructions
        i = 0
        while i < len(insts):
            inst = insts[i]
            si = inst.sync_info
            waits = list(si.on_wait) if si is not None and si.on_wait else []
            if len(waits) > cap:
                eng = nc.engines[inst.engine]
                excess, keep = waits[:-cap], waits[-cap:]
                si.on_wait = keep
                pos = i
                for j in range(0, len(excess), cap):
                    chunk = excess[j : j + cap]
                    ev = eng.wait_ge(dummy, 1)
                    cur_list = nc.cur_bb.bb.instructions
                    assert cur_list[-1] is ev.ins
                    cur_list.pop()
                    ev.ins.sync_info.on_wait = chunk
                    insts.insert(pos, ev.ins)
                    pos += 1
                    i += 1
            i += 1

    with nc.semaphore("waitfix_dummy") as dummy:
        for f in nc.m.functions:
            for bb in f.blocks:
                fix_block(bb, dummy)


def build_nc():
    nc = bass.Bass()
    xT_d = nc.declare_dram_parameter("xT", [8, 128, SEQ], BF16, isOutput=False)
    x8_d = nc.declare_dram_parameter("x8", [4, 128, 2, SEQ], FP8, isOutput=False)
    wq8_d = nc.declare_dram_parameter("wq8", [4, 128, 2, 256], FP8, isOutput=False)
    wk8_d = nc.declare_dram_parameter("wk8", [4, 128, 2, 256], FP8, isOutput=False)
    wv_d = nc.declare_dram_parameter("wv", [8, 128, GCOLS], BF16, isOutput=False)
    wo_d = nc.declare_dram_parameter("wo", [2, 128, EMB], BF16, isOutput=False)
    id_d = nc.declare_dram_parameter("ident", [128, 128], BF16, isOutput=False)
    y0_d = nc.declare_dram_parameter("y0", [8, 128, EMB], BF16, isOutput=True)
    y1_d = nc.declare_dram_parameter("y1", [8, 128, EMB], BF16, isOutput=True)

    with tile.TileContext(nc) as tc:
        with (
            tc.tile_pool(name="big", bufs=1) as big,
            tc.tile_pool(name="atp", bufs=1) as atp,
            tc.tile_pool(name="yst", bufs=8) as yst,
            tc.tile_pool(name="scp", bufs=2, space="PSUM") as scp,
            tc.tile_pool(name="ctxp", bufs=1, space="PSUM") as ctxp,
        ):
            # ---- SBUF ----
            XTk = [big.tile([128, SEQ], BF16, name=f"xt{k}") for k in range(8)]
            X8 = [big.tile([128, 2, SEQ], FP8, name=f"x8_{k}") for k in range(4)]
            WQ8 = big.tile([128, 4, 2, 256], FP8)
            WK8 = big.tile([128, 4, 2, 256], FP8)
            WV = big.tile([128, 8, GCOLS], BF16)
            WO = big.tile([128, 2, EMB], BF16)
            IDT = big.tile([128, 128], BF16)
            Q8v = [big.tile([128, 2, SEQ], FP8, name=f"q8v{v}") for v in range(4)]
            K8v = [big.tile([128, 2, SEQ], FP8, name=f"k8v{v}") for v in range(4)]
            VAr = [big.tile([128, 8, 8, 17], BF16, name=f"va{r}") for r in range(2)]
            CTr = [big.tile([128, 8, 128], BF16, name=f"ct{r}") for r in range(2)]
            CNr = [[big.tile([128, 128], BF16, name=f"cn{r}i{i}")
                    for i in range(8)] for r in range(2)]
            Y0 = big.tile([128, 8, EMB], BF16)
            JNK = big.tile([128, 512], BF16)
            AT = [[atp.tile([128, SPAN], BF16, name=f"at{i}s{s}")
                   for s in range(3)] for i in range(4)]

            scn = [0]

            def sc_tile(shape=None, dtype=F32, tag="sc"):
                scn[0] += 1
                pool = scp if tag == "sc" else ctxp
                return pool.tile(shape or [128, SPAN], dtype, tag=tag,
                                 name=f"{tag}{scn[0]}")

            def evict_copy(code, dst, src):
                if code == "A":
                    nc.scalar.copy(dst, src)
                else:
                    nc.vector.tensor_copy(dst, src)

            # ---- warm-up: ramp the PE pstate while DMAs stream ----
            nc.gpsimd.memset(JNK[:], 1.0)
            dmy = sc_tile([128, 512], F32, tag="tp")
            for _ in range(WARMUP_N):
                nc.tensor.matmul(dmy[:], JNK[:, 0:128], JNK[:],
                                 start=True, stop=True)

            # ---- DMA in: what gates Q/K projections first ----
            nc.sync.dma_start(WQ8[:], wq8_d[:].rearrange("k p a m -> p k a m"))
            nc.sync.dma_start(WK8[:], wk8_d[:].rearrange("k p a m -> p k a m"))
            for kbp in range(4):
                nc.sync.dma_start(X8[kbp][:], x8_d[kbp])
            nc.sync.dma_start(IDT[:], id_d[:])
            for r in range(2):
                nc.gpsimd.memset(VAr[r][:, :, :, 16:17], 1.0)

            # ---- Q/K projections: fp8 DoubleRow, contraction 256/ktile-pair
            evn = [0]

            def proj_qk(W8, DST, tags, ic_outer=False):
                combos = ([(t, ic) for ic in range(2) for t in range(2)]
                          if ic_outer else
                          [(t, ic) for t in range(2) for ic in range(2)])
                for t, ic in combos:
                    tg = tags[evn[0] % len(tags)]
                    if tg == "sc":
                        p = sc_tile()[:, 0:512]
                    else:
                        p = sc_tile([128, 512], F32, tag=tg)[:]
                    for kbp in range(4):
                        nc.tensor.matmul(
                            p, W8[:, kbp, :, 128 * t : 128 * t + 128],
                            X8[kbp][:, :, 512 * ic : 512 * ic + 512],
                            start=(kbp == 0), stop=(kbp == 3),
                            perf_mode=DR)
                    base = DST[:, t, 512 * ic : 512 * ic + 512]
                    # split the eviction across both engines
                    nc.scalar.copy(base[:, 0:256], p[:, 0:256])
                    nc.vector.tensor_copy(base[:, 256:512], p[:, 256:512])
                    evn[0] += 1

            proj_qk(WQ8, Q8v[0], ["sc"])
            proj_qk(WK8, K8v[0], ["tp", "ctx"], ic_outer=K_IC_OUTER)

            # ---- remaining input DMAs (no waits; keep SP queue flowing) ----
            for kb in range(8):
                nc.sync.dma_start(XTk[kb][:], xT_d[kb, :, :])
            nc.sync.dma_start(WV[:], wv_d[:].rearrange("k p n -> p k n"))
            nc.sync.dma_start(WO[:], wo_d[:].rearrange("r p n -> p r n"))

            # ---- shifted variants via partition-strided SBUF->SBUF DMA ----
            # (emitted after all input DMAs: their eviction waits would
            # head-of-line block the SP queue otherwise)
            for (SRC, DSTS) in ((Q8v[0], Q8v), (K8v[0], K8v)):
                s_u8 = SRC[:].bitcast(mybir.dt.uint8)
                for v in range(1, 4):
                    d_u8 = DSTS[v][:].bitcast(mybir.dt.uint8)
                    for k in range(4):
                        nc.sync.dma_start(
                            d_u8[32 * k : 32 * k + 8, :],
                            s_u8[32 * k + 8 * v : 32 * k + 8 * v + 8, :])

            # ---- deferred unit queue (popped between span exps) ----
            vcnt = [0]

            def proj_v_unit(mt, half):
                def f():
                    p = sc_tile([128, 512], F32, tag="tp")[:, 0:128]
                    for kb in range(8):
                        nc.tensor.matmul(
                            p, XTk[kb][:, 128 * mt : 128 * mt + 128],
                            WV[:, kb, 128 * half : 128 * half + 128],
                            start=(kb == 0), stop=(kb == 7))
                    code = V_EVICT[vcnt[0]]
                    vcnt[0] += 1
                    evict_copy(code,
                               VAr[half][:, mt, :, 0:16],
                               p.rearrange("p (h e) -> p h e", e=16))
                return f

            def transpose_unit(rb):
                def f():
                    TPb = sc_tile([128, 8, 128], BF16, tag="tp")
                    for ib in range(8):
                        nc.tensor.transpose(
                            TPb[:, ib, :], CTr[rb][:, ib, :], IDT[:])
                        evict_copy(CN0_EVICT[ib] if rb == 0 else CN1_EVICT[ib],
                                   CNr[rb][ib][:], TPb[:, ib, :])
                return f

            p0cnt = [0]

            def pass0_unit(ib, ic):
                def f():
                    tg = "tp" if (2 * ib + ic) % 2 == 0 else "ctx"
                    yp = sc_tile([128, 512], F32, tag=tg)[:]
                    nc.tensor.matmul(
                        yp, CNr[0][ib][:],
                        WO[:, 0, 512 * ic : 512 * ic + 512],
                        start=True, stop=True)
                    code = PASS0_EVICT[p0cnt[0]]
                    p0cnt[0] += 1
                    evict_copy(code,
                               Y0[:, ib, 512 * ic : 512 * ic + 512], yp)
                    nc.sync.dma_start(
                        y0_d[ib, :, 512 * ic : 512 * ic + 512],
                        Y0[:, ib, 512 * ic : 512 * ic + 512])
                return f

            units = []  # (min_head, thunk)
            for mt in range(8):
                units.append((0, proj_v_unit(mt, 0)))
            for mt in range(8):
                units.append((4, proj_v_unit(mt, 1)))
            units.append((9, transpose_unit(0)))
            for ib in range(8):
                for ic in range(2):
                    units.append((9, pass0_unit(ib, ic)))

            def head_slices(h, qk):
                v = h // 4
                b = 32 * (h % 4)
                return (Q8v if qk == "q" else K8v)[v], b

            # ---- attention ----
            def emit_mask(A, jb):
                o = SPAN_OF[jb][1]
                nc.gpsimd.affine_select(
                    out=A[:, o : o + 128], in_=A[:, o : o + 128],
                    compare_op=mybir.AluOpType.is_ge,
                    fill=0.0, base=0, pattern=[[1, 128]],
                    channel_multiplier=-1)

            def span_mms(h, s, sct):
                QT, b = head_slices(h, "q")
                KT, _ = head_slices(h, "k")
                for jb in SPAN_GROUPS[s]:
                    size = SIZES[jb]
                    i0 = 128 * jb
                    off = SPAN_OF[jb][1]
                    done = 0
                    while done < size:
                        # never cross a 512-f32 PSUM bank boundary
                        cw = min(512 - (off + done) % 512, size - done)
                        nc.tensor.matmul(
                            sct[:, off + done : off + done + cw],
                            KT[b : b + 8, :, i0 : i0 + 128],
                            QT[b : b + 8, :, i0 + done : i0 + done + cw],
                            start=True, stop=True,
                            perf_mode=DR, tile_position=(b, 0))
                        done += cw

            def span_exp(h, s, sct):
                A = AT[h % 4][s]
                if SPLIT_LAST_EXP and h == HPG - 1:
                    nc.scalar.activation(
                        A[:, 0:768], sct[:, 0:768],
                        mybir.ActivationFunctionType.Exp, scale=0.25)
                    nc.vector.tensor_scalar(
                        A[:, 768:SPAN].bitcast(I16), sct[:, 768:SPAN],
                        ACONST, BCONST,
                        mybir.AluOpType.mult, mybir.AluOpType.add)
                elif EXP_ENG[3 * h + s] == "A":
                    nc.scalar.activation(
                        A[:], sct[:],
                        mybir.ActivationFunctionType.Exp, scale=0.25)
                else:
                    nc.vector.tensor_scalar(
                        A[:].bitcast(I16), sct[:], ACONST, BCONST,
                        mybir.AluOpType.mult, mybir.AluOpType.add)
                for jb in SPAN_GROUPS[s]:
                    emit_mask(A, jb)

            def scores_and_exp(h):
                t0 = sc_tile()
                span_mms(h, 0, t0)
                t1 = sc_tile()
                span_mms(h, 1, t1)
                span_exp(h, 0, t0)
                t2 = sc_tile()
                span_mms(h, 2, t2)
                span_exp(h, 1, t1)
                for _ in range(4 if h <= 1 else 3):
                    if units and units[0][0] <= h:
                        units.pop(0)[1]()
                span_exp(h, 2, t2)

            def ctx_head(h):
                rb, hh = divmod(h, 8)
                CTX = ctxp.tile([128, 8, 17], F32, tag="ctx", name=f"ctx{h}")
                for ib in range(8):
                    # diag last so ctx starts before this head's masks finish
                    contribs = sorted((jb for jb in range(ib + 1)),
                                      key=lambda jb: ORDER_POS[jb])
                    if ib in contribs:
                        contribs.remove(ib)
                        contribs.append(ib)
                    for idx, jb in enumerate(contribs):
                        s, off = SPAN_OF[jb]
                        o = off + 128 * (ib - jb)
                        nc.tensor.matmul(
                            CTX[:, ib, :], AT[h % 4][s][:, o : o + 128],
                            VAr[rb][:, jb, hh, :],
                            start=(idx == 0), stop=(idx == len(contribs) - 1),
                            skip_group_check=True)
                R3 = yst.tile([128, 8], F32, tag="r3", bufs=2)
                nc.vector.reciprocal(
                    R3[:], CTX[:, :, 16:17].rearrange("p a b -> p (a b)"))
                nc.vector.tensor_mul(
                    CTr[rb][:, :, 16 * hh : 16 * hh + 16],
                    CTX[:, :, 0:16],
                    R3[:].unsqueeze(2).broadcast_to([128, 8, 16]))

            # ---- main loop ----
            for h in range(HPG):
                if h >= 2:
                    ctx_head(h - 2)
                scores_and_exp(h)
            ctx_head(HPG - 2)
            ctx_head(HPG - 1)

            # ---- out-projection pass 1 (ctx cols 128-255 partial),
            # fused per-ib: transpose -> CN evict -> matmuls -> Y evict ----
            if not FUSED_TAIL:
                transpose_unit(1)()
            else:
                TPc = sc_tile([128, 8, 128], BF16, tag="tp")
            for ib in range(8):
                if FUSED_TAIL:
                    nc.tensor.transpose(TPc[:, ib, :], CTr[1][:, ib, :], IDT[:])
                    evict_copy(CN1_EVICT[ib], CNr[1][ib][:], TPc[:, ib, :])
                yp = sc_tile()[:, 0:1024]
                for ic in range(2):
                    nc.tensor.matmul(
                        yp[:, 512 * ic : 512 * ic + 512],
                        CNr[1][ib][:],
                        WO[:, 1, 512 * ic : 512 * ic + 512],
                        start=True, stop=True)
                Y = yst.tile([128, 1024], BF16, tag="y", bufs=8)
                nc.scalar.copy(Y[:, 0:512], yp[:, 0:512])
                nc.vector.tensor_copy(Y[:, 512:1024], yp[:, 512:1024])
                nc.sync.dma_start(y1_d[ib, :, :], Y[:])

    split_excess_waits(nc)
    return nc


_NC_CACHE = None


def _get_nc():
    global _NC_CACHE
    if _NC_CACHE is None:
        _NC_CACHE = build_nc()
    return _NC_CACHE


def _bf(a):
    return np.ascontiguousarray(a).astype(ml_dtypes.bfloat16)


def _f8(a):
    return np.ascontiguousarray(a).astype(FP8NP)


def _colmap():
    """cm[128*t + m] = original column (within the 256-col group) stored
    at stationary free position m of chunk t, per the slot partition map."""
    cm = np.zeros(256, dtype=np.int64)
    for t in range(2):
        for m in range(128):
            v = (m % 32) // 8
            k = m // 32
            slot = 4 * v + k
            d = m % 8
            cm[128 * t + m] = 16 * slot + 8 * t + d
    return cm


_CM = _colmap()


def kernel(x, Wq, Wk, Wv, Wo, bo):
    x = np.asarray(x, dtype=np.float32)
    Wq = np.asarray(Wq, dtype=np.float32)
    Wk = np.asarray(Wk, dtype=np.float32)
    Wv = np.asarray(Wv, dtype=np.float32)
    Wo = np.asarray(Wo, dtype=np.float32)
    bo = np.asarray(bo, dtype=np.float32)

    nc = _get_nc()
    ident = np.eye(128, dtype=np.float32)

    in_maps = []
    for c in range(8):
        b, g = divmod(c, NG)
        cols = slice(GCOLS * g, GCOLS * g + GCOLS)
        xT = x[b].T  # [emb, seq]
        x8 = xT.reshape(4, 2, 128, SEQ).transpose(0, 2, 1, 3)
        # wq8[kbp, p, kt, 128*t+m] = W[256*kbp + 128*kt + p, g_base + cm[...]]
        wq8 = Wq[:, cols][:, _CM].reshape(4, 2, 128, 256).transpose(0, 2, 1, 3)
        wk8 = Wk[:, cols][:, _CM].reshape(4, 2, 128, 256).transpose(0, 2, 1, 3)
        in_maps.append({
            "xT": _bf(xT.reshape(8, 128, SEQ)),
            "x8": _f8(x8),
            "wq8": _f8(wq8),
            "wk8": _f8(wk8),
            "wv": _bf(Wv[:, cols].reshape(8, 128, GCOLS)),
            "wo": _bf(Wo[cols, :].reshape(2, 128, EMB)),
            "ident": _bf(ident),
        })

    res = run_bass_kernel_spmd(nc, in_maps, core_ids=list(range(8)))
    out = np.zeros((BATCH, SEQ, EMB), dtype=np.float32)
    for c in range(8):
        b = c // NG
        out[b] += res.results[c]["y0"].reshape(SEQ, EMB).astype(np.float32)
        out[b] += res.results[c]["y1"].reshape(SEQ, EMB).astype(np.float32)
    out += bo[None, None, :]
    return out


# revision 11
# speedup vs baseline: 1.0574x; 1.0248x over previous
"""Multi-head attention (axis-swapped variant) on 8 Trainium2 NeuronCores, v6.

Major changes over v5:
- Q/K projections computed as fp8(e4m3) DoubleRow matmuls (4x fewer PE
  cycles): host supplies x and Wq/Wk pre-quantized, with W columns
  permuted so each head's 8-dim slab lands on a 32-aligned partition
  base for its processing slot.
- Scores matmuls in fp8 DoubleRow (2x fewer PE cycles): Q/K stored as
  [128, 2(ktile), seq] fp8; misaligned head slabs served by shifted
  copies built with partition-strided SBUF->SBUF DMAs (no PE cost).
- exp split across ACT and DVE: ACT does exact Exp activations for most
  spans; DVE approximates exp for the rest via the Schraudolph
  int16-bitcast trick (tensor_scalar f32 -> int16 bits of the bf16
  result), validated bit-exact against the host model on the backend.
- Causal masks on Pool (gpsimd affine_select) for every head; Pool has
  no PSUM port so it carries no other work.
- PSUM evictions split between ACT (idle pre-attention / post-exp) and
  DVE by static tables.
"""

import numpy as np
import ml_dtypes

import concourse.bass as bass
import concourse.mybir as mybir
import concourse.tile as tile
from concourse.bass_utils import run_bass_kernel_spmd

F32 = mybir.dt.float32
BF16 = mybir.dt.bfloat16
FP8 = mybir.dt.float8e4
I16 = mybir.dt.int16
FP8NP = ml_dtypes.float8_e4m3

EMB = 1024
SEQ = 1024
BATCH = 2
NG = 4
HPG = 16
DH = 16
GCOLS = HPG * DH
DR = mybir.MatmulPerfMode.DoubleRow

SPAN = 1024  # sc slot width (f32), 2 PSUM banks, 3-deep rotation
NJB = 8

SIZES = [SEQ - 128 * jb for jb in range(NJB)]
SPAN_GROUPS = [[0], [1, 7], [2, 6], [3, 5], [4]]
WIDTHS = [sum(SIZES[jb] for jb in g) for g in SPAN_GROUPS]
assert all(w <= SPAN for w in WIDTHS) and sum(WIDTHS) == 4608
NS = len(SPAN_GROUPS)
SPAN_OF = {}
ORDER_POS = {}
_pos = 0
for _s, _g in enumerate(SPAN_GROUPS):
    _off = 0
    for _jb in _g:
        SPAN_OF[_jb] = (_s, _off)
        ORDER_POS[_jb] = _pos
        _pos += 1
        _off += SIZES[_jb]

# Schraudolph exp constants (bf16 bit-space), softmax scale 0.25 folded in.
ACONST = float(2.0 ** 7 / np.log(2.0) * 0.25)
BCONST = float(127.0 * 2 ** 7 - 0.0579 * 2 ** 7)

# ---- tuning tables -------------------------------------------------------
# exp engine per span index NS*h+s: 'A' = ACT exact, 'D' = DVE schraudolph.
EXP_A_FRAC = 0.68  # target ACT share of exp elements


def _exp_table():
    out = []
    done_a = 0.0
    done_tot = 0.0
    for j in range(16 * NS):
        w = WIDTHS[j % NS]
        done_tot += w
        if done_a + w <= EXP_A_FRAC * done_tot + 1e-9:
            out.append("A")
            done_a += w
        else:
            out.append("D")
    return out


EXP_ENG = _exp_table()
QK_EVICT = ["A", "A", "D", "A", "A", "A", "D", "A"]      # 8 x [128,512]
V_EVICT = ["D"] * 16                                      # 16 x [128,128]
CN0_EVICT = ["A", "D"] * 4                                # rb0 transposes
CN1_EVICT = ["A", "D"] * 4                                # rb1 (tail)
PASS0_EVICT = ["A", "D"] * 8                              # 16 x [128,512]
PASS1_EVICT = ["A", "D"] * 4                              # 8 x [128,1024]
K_IC_OUTER = False
PE_MASK_HEADS = frozenset({14, 15})  # diag masks folded into PE bias       # K projection ic-major (scores start earlier)
SPLIT_LAST_EXP = True   # last head's exps split across ACT+DVE
FUSED_TAIL = False       # per-ib transpose+CN+pass1 fusion
WARMUP_N = 2            # dummy warmup matmuls


def base_p(slot):
    """Q8/K8 partition base of the head processed in `slot` (0..15)."""
    return 32 * (slot % 4) + 8 * (slot // 4)


def split_excess_waits(nc, cap=1):
    def fix_block(bb, dummy):
        insts = bb.instructions
        i = 0
        while i < len(insts):
            inst = insts[i]
            si = inst.sync_info
            waits = list(si.on_wait) if si is not None and si.on_wait else []
            if len(waits) > cap:
                eng = nc.engines[inst.engine]
                excess, keep = waits[:-cap], waits[-cap:]
                si.on_wait = keep
                pos = i
                for j in range(0, len(excess), cap):
                    chunk = excess[j : j + cap]
                    ev = eng.wait_ge(dummy, 1)
                    cur_list = nc.cur_bb.bb.instructions
                    assert cur_list[-1] is ev.ins
                    cur_list.pop()
                    ev.ins.sync_info.on_wait = chunk
                    insts.insert(pos, ev.ins)
                    pos += 1
                    i += 1
            i += 1

    with nc.semaphore("waitfix_dummy") as dummy:
        for f in nc.m.functions:
            for bb in f.blocks:
                fix_block(bb, dummy)


def build_nc():
    nc = bass.Bass()
    xT_d = nc.declare_dram_parameter("xT", [8, 128, SEQ], BF16, isOutput=False)
    x8_d = nc.declare_dram_parameter("x8", [4, 128, 2, SEQ], FP8, isOutput=False)
    wq8_d = nc.declare_dram_parameter("wq8", [4, 128, 2, 256], FP8, isOutput=False)
    wk8_d = nc.declare_dram_parameter("wk8", [4, 128, 2, 256], FP8, isOutput=False)
    wv_d = nc.declare_dram_parameter("wv", [8, 128, GCOLS], BF16, isOutput=False)
    wo_d = nc.declare_dram_parameter("wo", [2, 128, EMB], BF16, isOutput=False)
    id_d = nc.declare_dram_parameter("ident", [128, 128], BF16, isOutput=False)
    mb_d = nc.declare_dram_parameter("mbias", [128, 128], BF16, isOutput=False)
    y0_d = nc.declare_dram_parameter("y0", [8, 128, EMB], BF16, isOutput=True)
    y1_d = nc.declare_dram_parameter("y1", [8, 128, EMB], BF16, isOutput=True)

    with tile.TileContext(nc) as tc:
        with (
            tc.tile_pool(name="big", bufs=1) as big,
            tc.tile_pool(name="atp", bufs=1) as atp,
            tc.tile_pool(name="yst", bufs=8) as yst,
            tc.tile_pool(name="scp", bufs=3, space="PSUM") as scp,
            tc.tile_pool(name="ctxp", bufs=1, space="PSUM") as ctxp,
        ):
            # ---- SBUF ----
            XTk = [big.tile([128, SEQ], BF16, name=f"xt{k}") for k in range(8)]
            X8 = [big.tile([128, 2, SEQ], FP8, name=f"x8_{k}") for k in range(4)]
            WQ8 = big.tile([128, 4, 2, 256], FP8)
            WK8 = big.tile([128, 4, 2, 256], FP8)
            WV = big.tile([128, 8, GCOLS], BF16)
            WO = big.tile([128, 2, EMB], BF16)
            IDT = big.tile([128, 128], BF16)
            MBIAS = big.tile([128, 128], BF16)
            Q8v = [big.tile([128, 2, SEQ], FP8, name=f"q8v{v}") for v in range(4)]
            K8v = [big.tile([128, 2, SEQ], FP8, name=f"k8v{v}") for v in range(4)]
            VAr = [big.tile([128, 8, 8, 17], BF16, name=f"va{r}") for r in range(2)]
            CTr = [big.tile([128, 8, 128], BF16, name=f"ct{r}") for r in range(2)]
            CNr = [[big.tile([128, 128], BF16, name=f"cn{r}i{i}")
                    for i in range(8)] for r in range(2)]
            Y0 = big.tile([128, 8, EMB], BF16)
            JNK = big.tile([128, 512], BF16)
            AT = [[atp.tile([128, SPAN], BF16, name=f"at{i}s{s}")
                   for s in range(NS)] for i in range(4)]

            scn = [0]

            def sc_tile(shape=None, dtype=F32, tag="sc"):
                scn[0] += 1
                pool = scp if tag == "sc" else ctxp
                return pool.tile(shape or [128, SPAN], dtype, tag=tag,
                                 name=f"{tag}{scn[0]}")

            def evict_copy(code, dst, src):
                if code == "A":
                    nc.scalar.copy(dst, src)
                else:
                    nc.vector.tensor_copy(dst, src)

            # ---- warm-up: ramp the PE pstate while DMAs stream ----
            nc.gpsimd.memset(JNK[:], 1.0)
            dmy = sc_tile([128, 512], F32, tag="tp")
            for _ in range(WARMUP_N):
                nc.tensor.matmul(dmy[:], JNK[:, 0:128], JNK[:],
                                 start=True, stop=True)

            # ---- DMA in: what gates Q/K projections first ----
            nc.sync.dma_start(WQ8[:], wq8_d[:].rearrange("k p a m -> p k a m"))
            nc.sync.dma_start(WK8[:], wk8_d[:].rearrange("k p a m -> p k a m"))
            for kbp in range(4):
                nc.sync.dma_start(X8[kbp][:], x8_d[kbp])
            nc.sync.dma_start(IDT[:], id_d[:])
            nc.sync.dma_start(MBIAS[:], mb_d[:])
            for r in range(2):
                nc.gpsimd.memset(VAr[r][:, :, :, 16:17], 1.0)

            # ---- Q/K projections: fp8 DoubleRow, contraction 256/ktile-pair
            evn = [0]

            def proj_qk_one(W8, DST, tg, t, ic):
                if tg == "sc":
                    p = sc_tile()[:, 0:512]
                else:
                    p = sc_tile([128, 512], F32, tag=tg)[:]
                for kbp in range(4):
                    nc.tensor.matmul(
                        p, W8[:, kbp, :, 128 * t : 128 * t + 128],
                        X8[kbp][:, :, 512 * ic : 512 * ic + 512],
                        start=(kbp == 0), stop=(kbp == 3),
                        perf_mode=DR)
                base = DST[:, t, 512 * ic : 512 * ic + 512]
                # split the eviction across both engines
                nc.scalar.copy(base[:, 0:256], p[:, 0:256])
                nc.vector.tensor_copy(base[:, 256:512], p[:, 256:512])
                evn[0] += 1

            for t in range(2):
                for ic in range(2):
                    proj_qk_one(WQ8, Q8v[0], "sc", t, ic)
                    proj_qk_one(WK8, K8v[0], ("tp", "ctx")[(2 * t + ic) % 2],
                                t, ic)

            # ---- remaining input DMAs (no waits; keep SP queue flowing) ----
            for kb in range(8):
                nc.sync.dma_start(XTk[kb][:], xT_d[kb, :, :])
            nc.sync.dma_start(WV[:], wv_d[:].rearrange("k p n -> p k n"))
            nc.sync.dma_start(WO[:], wo_d[:].rearrange("r p n -> p r n"))

            # ---- shifted variants via partition-strided SBUF->SBUF DMA ----
            # (emitted after all input DMAs: their eviction waits would
            # head-of-line block the SP queue otherwise)
            for (SRC, DSTS) in ((Q8v[0], Q8v), (K8v[0], K8v)):
                s_u8 = SRC[:].bitcast(mybir.dt.uint8)
                for v in range(1, 4):
                    d_u8 = DSTS[v][:].bitcast(mybir.dt.uint8)
                    for k in range(4):
                        nc.sync.dma_start(
                            d_u8[32 * k : 32 * k + 8, :],
                            s_u8[32 * k + 8 * v : 32 * k + 8 * v + 8, :])

            # ---- deferred unit queue (popped between span exps) ----
            vcnt = [0]

            def proj_v_unit(mt, half):
                def f():
                    p = sc_tile([128, 512], F32, tag="tp")[:, 0:128]
                    for kb in range(8):
                        nc.tensor.matmul(
                            p, XTk[kb][:, 128 * mt : 128 * mt + 128],
                            WV[:, kb, 128 * half : 128 * half + 128],
                            start=(kb == 0), stop=(kb == 7))
                    code = V_EVICT[vcnt[0]]
                    vcnt[0] += 1
                    evict_copy(code,
                               VAr[half][:, mt, :, 0:16],
                               p.rearrange("p (h e) -> p h e", e=16))
                return f

            def transpose_unit(rb):
                def f():
                    TPb = sc_tile([128, 8, 128], BF16, tag="tp")
                    for ib in range(8):
                        nc.tensor.transpose(
                            TPb[:, ib, :], CTr[rb][:, ib, :], IDT[:])
                        evict_copy(CN0_EVICT[ib] if rb == 0 else CN1_EVICT[ib],
                                   CNr[rb][ib][:], TPb[:, ib, :])
                return f

            p0cnt = [0]

            def pass0_unit(ib, ic):
                def f():
                    tg = "tp" if (2 * ib + ic) % 2 == 0 else "ctx"
                    yp = sc_tile([128, 512], F32, tag=tg)[:]
                    nc.tensor.matmul(
                        yp, CNr[0][ib][:],
                        WO[:, 0, 512 * ic : 512 * ic + 512],
                        start=True, stop=True)
                    code = PASS0_EVICT[p0cnt[0]]
                    p0cnt[0] += 1
                    evict_copy(code,
                               Y0[:, ib, 512 * ic : 512 * ic + 512], yp)
                    nc.sync.dma_start(
                        y0_d[ib, :, 512 * ic : 512 * ic + 512],
                        Y0[:, ib, 512 * ic : 512 * ic + 512])
                return f

            units = []  # (min_head, thunk)
            for mt in range(8):
                units.append((0, proj_v_unit(mt, 0)))
            for mt in range(8):
                units.append((4, proj_v_unit(mt, 1)))
            units.append((9, transpose_unit(0)))
            for ib in range(8):
                for ic in range(2):
                    units.append((9, pass0_unit(ib, ic)))

            def head_slices(h, qk):
                v = h // 4
                b = 32 * (h % 4)
                return (Q8v if qk == "q" else K8v)[v], b

            # ---- attention ----
            def emit_mask(A, jb):
                o = SPAN_OF[jb][1]
                nc.gpsimd.affine_select(
                    out=A[:, o : o + 128], in_=A[:, o : o + 128],
                    compare_op=mybir.AluOpType.is_ge,
                    fill=0.0, base=0, pattern=[[1, 128]],
                    channel_multiplier=-1)

            def span_mms(h, s, sct):
                QT, b = head_slices(h, "q")
                KT, _ = head_slices(h, "k")
                pe_mask = h in PE_MASK_HEADS
                for jb in SPAN_GROUPS[s]:
                    size = SIZES[jb]
                    i0 = 128 * jb
                    off = SPAN_OF[jb][1]
                    done = 0
                    while done < size:
                        # never cross a 512-f32 PSUM bank boundary
                        cw = min(512 - (off + done) % 512, size - done)
                        if pe_mask and done == 0:
                            cw = 128
                        diag_here = pe_mask and done == 0
                        nc.tensor.matmul(
                            sct[:, off + done : off + done + cw],
                            KT[b : b + 8, :, i0 : i0 + 128],
                            QT[b : b + 8, :, i0 + done : i0 + done + cw],
                            start=True, stop=not diag_here,
                            perf_mode=DR, tile_position=(b, 0))
                        if diag_here:
                            nc.tensor.matmul(
                                sct[:, off : off + 128],
                                IDT[:], MBIAS[:],
                                start=False, stop=True)
                        done += cw

            def span_exp(h, s, sct):
                A = AT[h % 4][s]
                W = WIDTHS[s]
                if SPLIT_LAST_EXP and h == HPG - 1:
                    wa = (W // 2 + 128) // 256 * 256
                    nc.scalar.activation(
                        A[:, 0:wa], sct[:, 0:wa],
                        mybir.ActivationFunctionType.Exp, scale=0.25)
                    nc.vector.tensor_scalar(
                        A[:, wa:W].bitcast(I16), sct[:, wa:W],
                        ACONST, BCONST,
                        mybir.AluOpType.mult, mybir.AluOpType.add)
                elif EXP_ENG[NS * h + s] == "A":
                    nc.scalar.activation(
                        A[:, 0:W], sct[:, 0:W],
                        mybir.ActivationFunctionType.Exp, scale=0.25)
                else:
                    nc.vector.tensor_scalar(
                        A[:, 0:W].bitcast(I16), sct[:, 0:W], ACONST, BCONST,
                        mybir.AluOpType.mult, mybir.AluOpType.add)
                if h not in PE_MASK_HEADS:
                    for jb in SPAN_GROUPS[s]:
                        emit_mask(A, jb)

            def scores_and_exp(h):
                tiles = [None] * NS

                def mms(s):
                    tiles[s] = sc_tile()
                    span_mms(h, s, tiles[s])

                def drain(n):
                    for _ in range(n):
                        if units and units[0][0] <= h:
                            units.pop(0)[1]()

                mms(0)
                mms(1)
                span_exp(h, 0, tiles[0])
                mms(2)
                span_exp(h, 1, tiles[1])
                drain(2 if h > 1 else 3)
                mms(3)
                span_exp(h, 2, tiles[2])
                mms(4)
                span_exp(h, 3, tiles[3])
                drain(1)
                span_exp(h, 4, tiles[4])

            def ctx_head(h):
                rb, hh = divmod(h, 8)
                CTX = ctxp.tile([128, 8, 17], F32, tag="ctx", name=f"ctx{h}")
                for ib in range(8):
                    # diag last so ctx starts before this head's masks finish
                    contribs = sorted((jb for jb in range(ib + 1)),
                                      key=lambda jb: ORDER_POS[jb])
                    if ib in contribs:
                        contribs.remove(ib)
                        contribs.append(ib)
                    for idx, jb in enumerate(contribs):
                        s, off = SPAN_OF[jb]
                        o = off + 128 * (ib - jb)
                        nc.tensor.matmul(
                            CTX[:, ib, :], AT[h % 4][s][:, o : o + 128],
                            VAr[rb][:, jb, hh, :],
                            start=(idx == 0), stop=(idx == len(contribs) - 1),
                            skip_group_check=True)
                R3 = yst.tile([128, 8], F32, tag="r3", bufs=2)
                nc.vector.reciprocal(
                    R3[:], CTX[:, :, 16:17].rearrange("p a b -> p (a b)"))
                nc.vector.tensor_mul(
                    CTr[rb][:, :, 16 * hh : 16 * hh + 16],
                    CTX[:, :, 0:16],
                    R3[:].unsqueeze(2).broadcast_to([128, 8, 16]))

            # ---- main loop ----
            for h in range(HPG):
                if h >= 2:
                    ctx_head(h - 2)
                scores_and_exp(h)
            ctx_head(HPG - 2)
            ctx_head(HPG - 1)

            # ---- out-projection pass 1 (ctx cols 128-255 partial),
            # fused per-ib: transpose -> CN evict -> matmuls -> Y evict ----
            if not FUSED_TAIL:
                transpose_unit(1)()
            else:
                TPc = sc_tile([128, 8, 128], BF16, tag="tp")
            for ib in range(8):
                if FUSED_TAIL:
                    nc.tensor.transpose(TPc[:, ib, :], CTr[1][:, ib, :], IDT[:])
                    evict_copy(CN1_EVICT[ib], CNr[1][ib][:], TPc[:, ib, :])
                yp = sc_tile()[:, 0:1024]
                for ic in range(2):
                    nc.tensor.matmul(
                        yp[:, 512 * ic : 512 * ic + 512],
                        CNr[1][ib][:],
                        WO[:, 1, 512 * ic : 512 * ic + 512],
                        start=True, stop=True)
                Y = yst.tile([128, 1024], BF16, tag="y", bufs=8)
                nc.scalar.copy(Y[:, 0:512], yp[:, 0:512])
                nc.vector.tensor_copy(Y[:, 512:1024], yp[:, 512:1024])
                nc.sync.dma_start(y1_d[ib, :, :], Y[:])

    split_excess_waits(nc)
    return nc


_NC_CACHE = None


def _get_nc():
    global _NC_CACHE
    if _NC_CACHE is None:
        _NC_CACHE = build_nc()
    return _NC_CACHE


def _bf(a):
    return np.ascontiguousarray(a).astype(ml_dtypes.bfloat16)


def _f8(a):
    return np.ascontiguousarray(a).astype(FP8NP)


def _colmap():
    """cm[128*t + m] = original column (within the 256-col group) stored
    at stationary free position m of chunk t, per the slot partition map."""
    cm = np.zeros(256, dtype=np.int64)
    for t in range(2):
        for m in range(128):
            v = (m % 32) // 8
            k = m // 32
            slot = 4 * v + k
            d = m % 8
            cm[128 * t + m] = 16 * slot + 8 * t + d
    return cm


_CM = _colmap()


def kernel(x, Wq, Wk, Wv, Wo, bo):
    x = np.asarray(x, dtype=np.float32)
    Wq = np.asarray(Wq, dtype=np.float32)
    Wk = np.asarray(Wk, dtype=np.float32)
    Wv = np.asarray(Wv, dtype=np.float32)
    Wo = np.asarray(Wo, dtype=np.float32)
    bo = np.asarray(bo, dtype=np.float32)

    nc = _get_nc()
    ident = np.eye(128, dtype=np.float32)

    in_maps = []
    for c in range(8):
        b, g = divmod(c, NG)
        cols = slice(GCOLS * g, GCOLS * g + GCOLS)
        xT = x[b].T  # [emb, seq]
        x8 = xT.reshape(4, 2, 128, SEQ).transpose(0, 2, 1, 3)
        # wq8[kbp, p, kt, 128*t+m] = W[256*kbp + 128*kt + p, g_base + cm[...]]
        wq8 = Wq[:, cols][:, _CM].reshape(4, 2, 128, 256).transpose(0, 2, 1, 3)
        wk8 = Wk[:, cols][:, _CM].reshape(4, 2, 128, 256).transpose(0, 2, 1, 3)
        in_maps.append({
            "xT": _bf(xT.reshape(8, 128, SEQ)),
            "x8": _f8(x8),
            "wq8": _f8(wq8),
            "wk8": _f8(wk8),
            "wv": _bf(Wv[:, cols].reshape(8, 128, GCOLS)),
            "wo": _bf(Wo[cols, :].reshape(2, 128, EMB)),
            "ident": _bf(ident),
        })

    res = run_bass_kernel_spmd(nc, in_maps, core_ids=list(range(8)))
    out = np.zeros((BATCH, SEQ, EMB), dtype=np.float32)
    for c in range(8):
        b = c // NG
        out[b] += res.results[c]["y0"].reshape(SEQ, EMB).astype(np.float32)
        out[b] += res.results[c]["y1"].reshape(SEQ, EMB).astype(np.float32)
    out += bo[None, None, :]
    return out


# revision 13
# speedup vs baseline: 1.0750x; 1.0166x over previous
"""Multi-head attention (axis-swapped variant) on 8 Trainium2 NeuronCores, v6.

Major changes over v5:
- Q/K projections computed as fp8(e4m3) DoubleRow matmuls (4x fewer PE
  cycles): host supplies x and Wq/Wk pre-quantized, with W columns
  permuted so each head's 8-dim slab lands on a 32-aligned partition
  base for its processing slot.
- Scores matmuls in fp8 DoubleRow (2x fewer PE cycles): Q/K stored as
  [128, 2(ktile), seq] fp8; misaligned head slabs served by shifted
  copies built with partition-strided SBUF->SBUF DMAs (no PE cost).
- exp split across ACT and DVE: ACT does exact Exp activations for most
  spans; DVE approximates exp for the rest via the Schraudolph
  int16-bitcast trick (tensor_scalar f32 -> int16 bits of the bf16
  result), validated bit-exact against the host model on the backend.
- Causal masks on Pool (gpsimd affine_select) for every head; Pool has
  no PSUM port so it carries no other work.
- PSUM evictions split between ACT (idle pre-attention / post-exp) and
  DVE by static tables.
"""

import numpy as np
import ml_dtypes

import concourse.bass as bass
import concourse.mybir as mybir
import concourse.tile as tile
from concourse.bass_utils import run_bass_kernel_spmd

F32 = mybir.dt.float32
BF16 = mybir.dt.bfloat16
FP8 = mybir.dt.float8e4
I16 = mybir.dt.int16
FP8NP = ml_dtypes.float8_e4m3

EMB = 1024
SEQ = 1024
BATCH = 2
NG = 4
HPG = 16
DH = 16
GCOLS = HPG * DH
DR = mybir.MatmulPerfMode.DoubleRow

SPAN = 1024  # sc slot width (f32), 2 PSUM banks, 3-deep rotation
NJB = 8

SIZES = [SEQ - 128 * jb for jb in range(NJB)]
SPAN_GROUPS = [[0], [1, 7], [2, 6], [3, 5], [4]]
WIDTHS = [sum(SIZES[jb] for jb in g) for g in SPAN_GROUPS]
assert all(w <= SPAN for w in WIDTHS) and sum(WIDTHS) == 4608
NS = len(SPAN_GROUPS)
SPAN_OF = {}
ORDER_POS = {}
_pos = 0
for _s, _g in enumerate(SPAN_GROUPS):
    _off = 0
    for _jb in _g:
        SPAN_OF[_jb] = (_s, _off)
        ORDER_POS[_jb] = _pos
        _pos += 1
        _off += SIZES[_jb]

# Schraudolph exp constants (bf16 bit-space), softmax scale 0.25 folded in.
ACONST = float(2.0 ** 7 / np.log(2.0) * 0.25)
BCONST = float(127.0 * 2 ** 7 - 0.0579 * 2 ** 7)

# ---- tuning tables -------------------------------------------------------
# exp engine per span index NS*h+s: 'A' = ACT exact, 'D' = DVE schraudolph.
EXP_A_FRAC = 0.68  # target ACT share of exp elements


def _exp_table():
    out = []
    done_a = 0.0
    done_tot = 0.0
    for j in range(16 * NS):
        w = WIDTHS[j % NS]
        done_tot += w
        if done_a + w <= EXP_A_FRAC * done_tot + 1e-9:
            out.append("A")
            done_a += w
        else:
            out.append("D")
    return out


EXP_ENG = _exp_table()
QK_EVICT = ["A", "A", "D", "A", "A", "A", "D", "A"]      # 8 x [128,512]
V_EVICT = ["D"] * 16                                      # 16 x [128,128]
CN0_EVICT = ["A", "D"] * 4                                # rb0 transposes
CN1_EVICT = ["A", "D"] * 4                                # rb1 (tail)
PASS0_EVICT = ["D"] * 16                                  # 16 x [128,512]
PASS1_EVICT = ["A", "D"] * 4                              # 8 x [128,1024]
K_IC_OUTER = False
PE_MASK_HEADS = frozenset({15})  # diag masks folded into PE bias       # K projection ic-major (scores start earlier)
SPLIT_LAST_EXP = True   # last head's exps split across ACT+DVE
FUSED_TAIL = False       # per-ib transpose+CN+pass1 fusion
WARMUP_N = 2            # dummy warmup matmuls


def base_p(slot):
    """Q8/K8 partition base of the head processed in `slot` (0..15)."""
    return 32 * (slot % 4) + 8 * (slot // 4)


def split_excess_waits(nc, cap=1):
    def fix_block(bb, dummy):
        insts = bb.instructions
        i = 0
        while i < len(insts):
            inst = insts[i]
            si = inst.sync_info
            waits = list(si.on_wait) if si is not None and si.on_wait else []
            if len(waits) > cap:
                eng = nc.engines[inst.engine]
                excess, keep = waits[:-cap], waits[-cap:]
                si.on_wait = keep
                pos = i
                for j in range(0, len(excess), cap):
                    chunk = excess[j : j + cap]
                    ev = eng.wait_ge(dummy, 1)
                    cur_list = nc.cur_bb.bb.instructions
                    assert cur_list[-1] is ev.ins
                    cur_list.pop()
                    ev.ins.sync_info.on_wait = chunk
                    insts.insert(pos, ev.ins)
                    pos += 1
                    i += 1
            i += 1

    with nc.semaphore("waitfix_dummy") as dummy:
        for f in nc.m.functions:
            for bb in f.blocks:
                fix_block(bb, dummy)


def build_nc():
    nc = bass.Bass()
    xT_d = nc.declare_dram_parameter("xT", [8, 128, SEQ], BF16, isOutput=False)
    x8_d = nc.declare_dram_parameter("x8", [4, 128, 2, SEQ], FP8, isOutput=False)
    wq8_d = nc.declare_dram_parameter("wq8", [4, 128, 2, 256], FP8, isOutput=False)
    wk8_d = nc.declare_dram_parameter("wk8", [4, 128, 2, 256], FP8, isOutput=False)
    wv_d = nc.declare_dram_parameter("wv", [8, 128, GCOLS], BF16, isOutput=False)
    wo_d = nc.declare_dram_parameter("wo", [2, 128, EMB], BF16, isOutput=False)
    id_d = nc.declare_dram_parameter("ident", [128, 128], BF16, isOutput=False)
    mb_d = nc.declare_dram_parameter("mbias", [128, 128], BF16, isOutput=False)
    y0_d = nc.declare_dram_parameter("y0", [8, 128, EMB], BF16, isOutput=True)
    y1_d = nc.declare_dram_parameter("y1", [8, 128, EMB], BF16, isOutput=True)

    with tile.TileContext(nc) as tc:
        with (
            tc.tile_pool(name="big", bufs=1) as big,
            tc.tile_pool(name="atp", bufs=1) as atp,
            tc.tile_pool(name="yst", bufs=8) as yst,
            tc.tile_pool(name="scp", bufs=3, space="PSUM") as scp,
            tc.tile_pool(name="ctxp", bufs=1, space="PSUM") as ctxp,
        ):
            # ---- SBUF ----
            XTk = [big.tile([128, SEQ], BF16, name=f"xt{k}") for k in range(8)]
            X8 = [big.tile([128, 2, SEQ], FP8, name=f"x8_{k}") for k in range(4)]
            WQ8 = big.tile([128, 4, 2, 256], FP8)
            WK8 = big.tile([128, 4, 2, 256], FP8)
            WV = big.tile([128, 8, GCOLS], BF16)
            WO = big.tile([128, 2, EMB], BF16)
            IDT = big.tile([128, 128], BF16)
            MBIAS = big.tile([128, 128], BF16)
            Q8v = [big.tile([128, 2, SEQ], FP8, name=f"q8v{v}") for v in range(4)]
            K8v = [big.tile([128, 2, SEQ], FP8, name=f"k8v{v}") for v in range(4)]
            VAr = [big.tile([128, 8, 8, 17], BF16, name=f"va{r}") for r in range(2)]
            CTr = [big.tile([128, 8, 128], BF16, name=f"ct{r}") for r in range(2)]
            CNr = [[big.tile([128, 128], BF16, name=f"cn{r}i{i}")
                    for i in range(8)] for r in range(2)]
            Y0 = big.tile([128, 8, EMB], BF16)
            JNK = big.tile([128, 512], BF16)
            AT = [[atp.tile([128, SPAN], BF16, name=f"at{i}s{s}")
                   for s in range(NS)] for i in range(4)]

            scn = [0]

            def sc_tile(shape=None, dtype=F32, tag="sc"):
                scn[0] += 1
                pool = scp if tag == "sc" else ctxp
                return pool.tile(shape or [128, SPAN], dtype, tag=tag,
                                 name=f"{tag}{scn[0]}")

            def evict_copy(code, dst, src):
                if code == "A":
                    nc.scalar.copy(dst, src)
                else:
                    nc.vector.tensor_copy(dst, src)

            # ---- warm-up: ramp the PE pstate while DMAs stream ----
            nc.gpsimd.memset(JNK[:], 1.0)
            dmy = sc_tile([128, 512], F32, tag="tp")
            for _ in range(WARMUP_N):
                nc.tensor.matmul(dmy[:], JNK[:, 0:128], JNK[:],
                                 start=True, stop=True)

            # ---- DMA in: what gates Q/K projections first ----
            nc.sync.dma_start(WQ8[:], wq8_d[:].rearrange("k p a m -> p k a m"))
            nc.sync.dma_start(WK8[:], wk8_d[:].rearrange("k p a m -> p k a m"))
            for kbp in range(4):
                nc.sync.dma_start(X8[kbp][:], x8_d[kbp])
            nc.sync.dma_start(IDT[:], id_d[:])
            nc.sync.dma_start(MBIAS[:], mb_d[:])
            for r in range(2):
                nc.gpsimd.memset(VAr[r][:, :, :, 16:17], 1.0)

            # ---- Q/K projections: fp8 DoubleRow, contraction 256/ktile-pair
            evn = [0]

            def proj_qk_one(W8, DST, tg, t, ic):
                if tg == "sc":
                    p = sc_tile()[:, 0:512]
                else:
                    p = sc_tile([128, 512], F32, tag=tg)[:]
                for kbp in range(4):
                    nc.tensor.matmul(
                        p, W8[:, kbp, :, 128 * t : 128 * t + 128],
                        X8[kbp][:, :, 512 * ic : 512 * ic + 512],
                        start=(kbp == 0), stop=(kbp == 3),
                        perf_mode=DR)
                base = DST[:, t, 512 * ic : 512 * ic + 512]
                # split the eviction across both engines
                nc.scalar.copy(base[:, 0:256], p[:, 0:256])
                nc.vector.tensor_copy(base[:, 256:512], p[:, 256:512])
                evn[0] += 1

            for t in range(2):
                for ic in range(2):
                    proj_qk_one(WQ8, Q8v[0], "sc", t, ic)
                    proj_qk_one(WK8, K8v[0], ("tp", "ctx")[(2 * t + ic) % 2],
                                t, ic)

            # ---- remaining input DMAs (no waits; keep SP queue flowing) ----
            for kb in range(8):
                nc.sync.dma_start(XTk[kb][:], xT_d[kb, :, :])
            nc.sync.dma_start(WV[:], wv_d[:].rearrange("k p n -> p k n"))
            nc.sync.dma_start(WO[:], wo_d[:].rearrange("r p n -> p r n"))

            # ---- shifted variants via partition-strided SBUF->SBUF DMA ----
            # (emitted after all input DMAs: their eviction waits would
            # head-of-line block the SP queue otherwise)
            for (SRC, DSTS) in ((Q8v[0], Q8v), (K8v[0], K8v)):
                s_u8 = SRC[:].bitcast(mybir.dt.uint8)
                for v in range(1, 4):
                    d_u8 = DSTS[v][:].bitcast(mybir.dt.uint8)
                    for k in range(4):
                        nc.sync.dma_start(
                            d_u8[32 * k : 32 * k + 8, :],
                            s_u8[32 * k + 8 * v : 32 * k + 8 * v + 8, :])

            # ---- deferred unit queue (popped between span exps) ----
            vcnt = [0]

            def proj_v_unit(mt, half):
                def f():
                    p = sc_tile([128, 512], F32, tag="tp")[:, 0:128]
                    for kb in range(8):
                        nc.tensor.matmul(
                            p, XTk[kb][:, 128 * mt : 128 * mt + 128],
                            WV[:, kb, 128 * half : 128 * half + 128],
                            start=(kb == 0), stop=(kb == 7))
                    code = V_EVICT[vcnt[0]]
                    vcnt[0] += 1
                    evict_copy(code,
                               VAr[half][:, mt, :, 0:16],
                               p.rearrange("p (h e) -> p h e", e=16))
                return f

            def transpose_unit(rb):
                def f():
                    TPb = sc_tile([128, 8, 128], BF16, tag="tp")
                    for ib in range(8):
                        nc.tensor.transpose(
                            TPb[:, ib, :], CTr[rb][:, ib, :], IDT[:])
                        evict_copy(CN0_EVICT[ib] if rb == 0 else CN1_EVICT[ib],
                                   CNr[rb][ib][:], TPb[:, ib, :])
                return f

            p0cnt = [0]

            def pass0_unit(ib, ic):
                def f():
                    tg = "tp" if (2 * ib + ic) % 2 == 0 else "ctx"
                    yp = sc_tile([128, 512], F32, tag=tg)[:]
                    nc.tensor.matmul(
                        yp, CNr[0][ib][:],
                        WO[:, 0, 512 * ic : 512 * ic + 512],
                        start=True, stop=True)
                    code = PASS0_EVICT[p0cnt[0]]
                    p0cnt[0] += 1
                    evict_copy(code,
                               Y0[:, ib, 512 * ic : 512 * ic + 512], yp)
                    nc.sync.dma_start(
                        y0_d[ib, :, 512 * ic : 512 * ic + 512],
                        Y0[:, ib, 512 * ic : 512 * ic + 512])
                return f

            units = []  # (min_head, thunk)
            for mt in range(8):
                units.append((0, proj_v_unit(mt, 0)))
            for mt in range(8):
                units.append((4, proj_v_unit(mt, 1)))
            units.append((9, transpose_unit(0)))
            for ib in range(8):
                for ic in range(2):
                    units.append((9, pass0_unit(ib, ic)))

            def head_slices(h, qk):
                v = h // 4
                b = 32 * (h % 4)
                return (Q8v if qk == "q" else K8v)[v], b

            # ---- attention ----
            def emit_mask(A, jb):
                o = SPAN_OF[jb][1]
                nc.gpsimd.affine_select(
                    out=A[:, o : o + 128], in_=A[:, o : o + 128],
                    compare_op=mybir.AluOpType.is_ge,
                    fill=0.0, base=0, pattern=[[1, 128]],
                    channel_multiplier=-1)

            def span_mms(h, s, sct):
                QT, b = head_slices(h, "q")
                KT, _ = head_slices(h, "k")
                pe_mask = h in PE_MASK_HEADS
                for jb in SPAN_GROUPS[s]:
                    size = SIZES[jb]
                    i0 = 128 * jb
                    off = SPAN_OF[jb][1]
                    done = 0
                    while done < size:
                        # never cross a 512-f32 PSUM bank boundary
                        cw = min(512 - (off + done) % 512, size - done)
                        if pe_mask and done == 0:
                            cw = 128
                        diag_here = pe_mask and done == 0
                        nc.tensor.matmul(
                            sct[:, off + done : off + done + cw],
                            KT[b : b + 8, :, i0 : i0 + 128],
                            QT[b : b + 8, :, i0 + done : i0 + done + cw],
                            start=True, stop=not diag_here,
                            perf_mode=DR, tile_position=(b, 0))
                        if diag_here:
                            nc.tensor.matmul(
                                sct[:, off : off + 128],
                                IDT[:], MBIAS[:],
                                start=False, stop=True)
                        done += cw

            def span_exp(h, s, sct):
                A = AT[h % 4][s]
                W = WIDTHS[s]
                if SPLIT_LAST_EXP and h == HPG - 1:
                    wa = (W // 2 + 128) // 256 * 256
                    nc.scalar.activation(
                        A[:, 0:wa], sct[:, 0:wa],
                        mybir.ActivationFunctionType.Exp, scale=0.25)
                    nc.vector.tensor_scalar(
                        A[:, wa:W].bitcast(I16), sct[:, wa:W],
                        ACONST, BCONST,
                        mybir.AluOpType.mult, mybir.AluOpType.add)
                elif EXP_ENG[NS * h + s] == "A":
                    nc.scalar.activation(
                        A[:, 0:W], sct[:, 0:W],
                        mybir.ActivationFunctionType.Exp, scale=0.25)
                else:
                    nc.vector.tensor_scalar(
                        A[:, 0:W].bitcast(I16), sct[:, 0:W], ACONST, BCONST,
                        mybir.AluOpType.mult, mybir.AluOpType.add)
                if h not in PE_MASK_HEADS:
                    for jb in SPAN_GROUPS[s]:
                        emit_mask(A, jb)

            def scores_and_exp(h):
                tiles = [None] * NS

                def mms(s):
                    tiles[s] = sc_tile()
                    span_mms(h, s, tiles[s])

                def drain(n):
                    for _ in range(n):
                        if units and units[0][0] <= h:
                            units.pop(0)[1]()

                mms(0)
                mms(1)
                span_exp(h, 0, tiles[0])
                mms(2)
                span_exp(h, 1, tiles[1])
                drain(2 if h > 1 else 3)
                mms(3)
                span_exp(h, 2, tiles[2])
                mms(4)
                span_exp(h, 3, tiles[3])
                drain(1)
                span_exp(h, 4, tiles[4])

            def ctx_head(h):
                rb, hh = divmod(h, 8)
                CTX = ctxp.tile([128, 8, 17], F32, tag="ctx", name=f"ctx{h}")
                for ib in range(8):
                    # diag last so ctx starts before this head's masks finish
                    contribs = sorted((jb for jb in range(ib + 1)),
                                      key=lambda jb: ORDER_POS[jb])
                    if ib in contribs:
                        contribs.remove(ib)
                        contribs.append(ib)
                    for idx, jb in enumerate(contribs):
                        s, off = SPAN_OF[jb]
                        o = off + 128 * (ib - jb)
                        nc.tensor.matmul(
                            CTX[:, ib, :], AT[h % 4][s][:, o : o + 128],
                            VAr[rb][:, jb, hh, :],
                            start=(idx == 0), stop=(idx == len(contribs) - 1),
                            skip_group_check=True)
                R3 = yst.tile([128, 8], F32, tag="r3", bufs=2)
                nc.vector.reciprocal(
                    R3[:], CTX[:, :, 16:17].rearrange("p a b -> p (a b)"))
                nc.vector.tensor_mul(
                    CTr[rb][:, :, 16 * hh : 16 * hh + 16],
                    CTX[:, :, 0:16],
                    R3[:].unsqueeze(2).broadcast_to([128, 8, 16]))

            # ---- main loop ----
            for h in range(HPG):
                if h >= 2:
                    ctx_head(h - 2)
                scores_and_exp(h)
            ctx_head(HPG - 2)
            ctx_head(HPG - 1)

            # ---- out-projection pass 1 (ctx cols 128-255 partial),
            # fused per-ib: transpose -> CN evict -> matmuls -> Y evict ----
            if not FUSED_TAIL:
                transpose_unit(1)()
            else:
                TPc = sc_tile([128, 8, 128], BF16, tag="tp")
            for ib in range(8):
                if FUSED_TAIL:
                    nc.tensor.transpose(TPc[:, ib, :], CTr[1][:, ib, :], IDT[:])
                    evict_copy(CN1_EVICT[ib], CNr[1][ib][:], TPc[:, ib, :])
                yp = sc_tile()[:, 0:1024]
                for ic in range(2):
                    nc.tensor.matmul(
                        yp[:, 512 * ic : 512 * ic + 512],
                        CNr[1][ib][:],
                        WO[:, 1, 512 * ic : 512 * ic + 512],
                        start=True, stop=True)
                Y = yst.tile([128, 1024], BF16, tag="y", bufs=8)
                nc.scalar.copy(Y[:, 0:512], yp[:, 0:512])
                nc.vector.tensor_copy(Y[:, 512:1024], yp[:, 512:1024])
                nc.sync.dma_start(y1_d[ib, :, :], Y[:])

    split_excess_waits(nc)
    return nc


_NC_CACHE = None


def _get_nc():
    global _NC_CACHE
    if _NC_CACHE is None:
        _NC_CACHE = build_nc()
    return _NC_CACHE


def _bf(a):
    return np.ascontiguousarray(a).astype(ml_dtypes.bfloat16)


def _f8(a):
    return np.ascontiguousarray(a).astype(FP8NP)


def _colmap():
    """cm[128*t + m] = original column (within the 256-col group) stored
    at stationary free position m of chunk t, per the slot partition map."""
    cm = np.zeros(256, dtype=np.int64)
    for t in range(2):
        for m in range(128):
            v = (m % 32) // 8
            k = m // 32
            slot = 4 * v + k
            d = m % 8
            cm[128 * t + m] = 16 * slot + 8 * t + d
    return cm


_CM = _colmap()


def kernel(x, Wq, Wk, Wv, Wo, bo):
    x = np.asarray(x, dtype=np.float32)
    Wq = np.asarray(Wq, dtype=np.float32)
    Wk = np.asarray(Wk, dtype=np.float32)
    Wv = np.asarray(Wv, dtype=np.float32)
    Wo = np.asarray(Wo, dtype=np.float32)
    bo = np.asarray(bo, dtype=np.float32)

    nc = _get_nc()
    ident = np.eye(128, dtype=np.float32)
    mbias = np.tril(np.ones((128, 128), dtype=np.float32), -1) * -200.0

    in_maps = []
    for c in range(8):
        b, g = divmod(c, NG)
        cols = slice(GCOLS * g, GCOLS * g + GCOLS)
        xT = x[b].T  # [emb, seq]
        x8 = xT.reshape(4, 2, 128, SEQ).transpose(0, 2, 1, 3)
        # wq8[kbp, p, kt, 128*t+m] = W[256*kbp + 128*kt + p, g_base + cm[...]]
        wq8 = Wq[:, cols][:, _CM].reshape(4, 2, 128, 256).transpose(0, 2, 1, 3)
        wk8 = Wk[:, cols][:, _CM].reshape(4, 2, 128, 256).transpose(0, 2, 1, 3)
        in_maps.append({
            "xT": _bf(xT.reshape(8, 128, SEQ)),
            "x8": _f8(x8),
            "wq8": _f8(wq8),
            "wk8": _f8(wk8),
            "wv": _bf(Wv[:, cols].reshape(8, 128, GCOLS)),
            "wo": _bf(Wo[cols, :].reshape(2, 128, EMB)),
            "ident": _bf(ident),
            "mbias": _bf(mbias),
        })

    res = run_bass_kernel_spmd(nc, in_maps, core_ids=list(range(8)))
    out = np.zeros((BATCH, SEQ, EMB), dtype=np.float32)
    for c in range(8):
        b = c // NG
        out[b] += res.results[c]["y0"].reshape(SEQ, EMB).astype(np.float32)
        out[b] += res.results[c]["y1"].reshape(SEQ, EMB).astype(np.float32)
    out += bo[None, None, :]
    return out
